# revision 34
# baseline (speedup 1.0000x reference)
"""DRN layer kernel for 8 TRN2 NeuronCores (4-way group-sum + fp8 DoubleRow).

Math (reference):
    T[j,k,l,m]   = exp(-w[j,k] * (s0[m]-s1[l])^2)
    Pw[i,j,k,l]  = sum_m T[j,k,l,m] * P[i,k,m]
    logsum[i,j,l]= sum_k log(Pw[i,j,k,l])
    out          = softmax_l(logsum + exponent_B[j,l])

With P' = P/S and t' = T - 1:  log Pw = log S + log1p(r),
r = sum_m t' P', |r| <= 0.105. log S cancels in the softmax.

Group-sum approximation: sum_{k in G} log1p(r_k) ~= log1p(sum r_k)
for groups of 4 ks (greedy weights-only matching minimizes the dropped
cross terms). Each group's R = sum of 4 r_k comes out of ONE fp8
DoubleRow matmul (256 contraction slots = 4x64 m-rows packed
2-per-cell): 32 MMs per core.

Device does ONLY the matmuls and per-tile consumption:
  c0/c1: DVE product chains     chain = (R + 1) * chain
  g:     ScalarE log1p -> f32, GpSimd adds into an SBUF accumulator
  s:     ScalarE log1p -> fp16, DMA straight to DRAM
exp / exponent_B / softmax normalization run on the HOST (f64), so the
device has no tail: the last consumer op is followed only by its
output DMA. All input DMAs are issued upfront on the sync queue (the
whole 3MB input is SBUF-resident), so no engine queue blocks the feed.

Sharding: tensor-parallel over n_upper: 8 cores x 8 upper nodes, full
batch per core.
"""

import numpy as np

B, NU, NL, QU, QL = 256, 64, 64, 64, 64
NCORES = 8
JLOC = NU // NCORES  # 8 upper nodes per core
JL = JLOC * QU       # 512 = packed (j, l) free dim
NGRP = NL // 4       # 16 k-groups of 4
KP2 = 128            # DoubleRow: 256 contraction slots as [128 part, 2]
PWK = B + JL         # 768 packed width per group: [P'^T (256 i) | t' (512)]
NKB = NGRP // 2      # 8 two-group DMA blocks

# route per group-tile: c0/c1 = DVE product chains, g = ScalarE log +
# GpSimd accumulate (first g writes acc directly), s = ScalarE log ->
# fp16 -> DMA out (host sums the logs). The last tile is 's' so the
# final consumer is the (fast) ScalarE, not the backlogged DVE.
ROUTE = ["g", "c0", "s", "g", "c1", "c0", "g", "c1",
         "c0", "s", "c1", "c0", "s", "c1", "c0", "s"]
NS = ROUTE.count("s")
assert len(ROUTE) == NGRP


def _build_program():
    import concourse.bass as bass
    import concourse.bacc as bacc
    import concourse.mybir as mybir
    from concourse.tile import TileContext

    f32 = mybir.dt.float32
    f16 = mybir.dt.float16
    fp8 = mybir.dt.float8e4
    AF = mybir.ActivationFunctionType
    ALU = mybir.AluOpType

    nc = bacc.Bacc(None, target_bir_lowering=False)
    # raw (non-pool) SBUF scratch for PE warm-up matmuls: deliberately
    # never written -- the values are never consumed, and skipping the
    # memset lets the warm-ups start at tensor-queue boot
    wsrc = nc.alloc_sbuf_tensor("warmsrc", [128, 1280], fp8)
    PTT = nc.declare_dram_parameter("PTT", [NKB, KP2, 4 * PWK], fp8,
                                    isOutput=False)
    CH = nc.declare_dram_parameter("ch", [2, 128, 2 * JL], f16, isOutput=True)
    ACC = nc.declare_dram_parameter("acc", [128, 2 * JL], f16, isOutput=True)
    LG = nc.declare_dram_parameter("lg", [NS, 128, 2 * JL], f16,
                                   isOutput=True)

    with TileContext(nc) as tc:
        with (
            tc.tile_pool(name="pth", bufs=1) as hpool,
            tc.tile_pool(name="ps", bufs=4, space="PSUM") as pspool,
            tc.tile_pool(name="lgw", bufs=2) as lwpool,
            tc.tile_pool(name="lgb", bufs=3) as lbpool,
            tc.tile_pool(name="ch", bufs=1) as chpool,
        ):
            chains = {
                "c0": chpool.tile([128, 2 * JL], f32, tag="ch0", name="ch0"),
                "c1": chpool.tile([128, 2 * JL], f32, tag="ch1", name="ch1"),
            }
            chf = {
                "c0": chpool.tile([128, 2 * JL], f16, tag="cf0", name="cf0"),
                "c1": chpool.tile([128, 2 * JL], f16, tag="cf1", name="cf1"),
            }
            acc = chpool.tile([128, 2 * JL], f32, tag="acc", name="acc")

            # all input DMAs upfront, in ascending-size chunks (in
            # groups: 1,1,2,4,4,4). Small chunks first so the stream
            # starts on the first 192KB; big chunks later mean fewer
            # DMA-completion semaphore waits (~0.5us lag each) at
            # chunk boundaries mid-stream.
            srcs = []  # per group p: (tile, col offset)
            ha = hpool.tile([KP2, 2 * PWK], fp8, tag="ptta")
            nc.sync.dma_start(out=ha[:], in_=PTT[0, :, :2 * PWK])
            hb = hpool.tile([KP2, 2 * PWK], fp8, tag="pttb")
            nc.sync.dma_start(out=hb[:], in_=PTT[0, :, 2 * PWK:])
            srcs += [(ha, 0), (hb, 0)]
            for kb in range(1, NKB):
                ptt = hpool.tile([KP2, 4 * PWK], fp8, tag=f"ptt{kb}")
                # late blocks trigger from the scalar HWDGE so the two
                # DGEs generate descriptors in parallel
                dge = nc.sync if kb <= 2 else nc.scalar
                dge.dma_start(out=ptt[:], in_=PTT[kb])
                srcs += [(ptt, 0), (ptt, 2 * PWK)]

            # PE pstate warm-up: the tensor engine only reaches its full
            # clock after ~3us of sustained execution, so burn the DMA
            # pre-roll on dummy matmuls. Their PSUM tile is a regular
            # ring slot reused by the real loop.
            wv = wsrc.ap().rearrange("q (c w) -> q c w", c=2)
            warm = pspool.tile([128, 2 * JL], f32, tag="ps", name="ps")
            for _ in range(8):
                nc.tensor.matmul(
                    warm[:, :JL], lhsT=wv[:, :, :128], rhs=wv[:, :, 128:],
                    start=True, stop=True,
                    perf_mode=mybir.MatmulPerfMode.DoubleRow)

            started = {"c0": False, "c1": False, "g": False}
            si = 0
            exports = []
            for p in range(NGRP):
                tile, off = srcs[p]
                pk = tile[:, off:off + 2 * PWK].rearrange(
                    "q (c w) -> q c w", c=2)
                ps = pspool.tile([128, 2 * JL], f32, tag="ps", name="ps")
                for ih in range(2):
                    nc.tensor.matmul(
                        ps[:, ih * JL:(ih + 1) * JL],
                        lhsT=pk[:, :, ih * 128:(ih + 1) * 128],
                        rhs=pk[:, :, B:PWK],
                        start=True, stop=True,
                        perf_mode=mybir.MatmulPerfMode.DoubleRow)
                r = ROUTE[p]
                if r in ("c0", "c1"):
                    ch = chains[r]
                    last = p == max(i for i, x in enumerate(ROUTE) if x == r)
                    if not started[r]:
                        nc.vector.tensor_scalar_add(ch[:], ps[:], 1.0)
                        started[r] = True
                    elif last:  # final factor writes the f16 export tile
                        nc.vector.scalar_tensor_tensor(
                            chf[r][:], ps[:], 1.0, ch[:],
                            op0=ALU.add, op1=ALU.mult)
                    else:
                        nc.vector.scalar_tensor_tensor(
                            ch[:], ps[:], 1.0, ch[:],
                            op0=ALU.add, op1=ALU.mult)
                elif r == "g":
                    if not started["g"]:
                        nc.scalar.activation(acc[:], ps[:], AF.Ln, bias=1.0)
                        started["g"] = True
                    else:
                        lgw = lwpool.tile([128, 2 * JL], f32, tag="lgw",
                                          name="lgw")
                        nc.scalar.activation(lgw[:], ps[:], AF.Ln, bias=1.0)
                        nc.gpsimd.tensor_add(acc[:], acc[:], lgw[:])
                else:  # s: log1p -> fp16 -> DRAM, host sums
                    lgb = lbpool.tile([128, 2 * JL], f16, tag="lgb",
                                      name="lgb")
                    nc.scalar.activation(lgb[:], ps[:], AF.Ln, bias=1.0)
                    nc.scalar.dma_start(out=LG[si], in_=lgb[:])
                    si += 1

            # Exports NEVER go on the sync queue: the SP sequencer also
            # processes every DMA-completion semaphore event in order, so
            # a waiting export trigger would head-of-line block the
            # events the matmul stream depends on. acc + c0 ride the
            # gpsimd SWDGE (idle; acc is cast f32->f16), c1 rides scalar
            # after the last LN.
            nc.gpsimd.dma_start(out=ACC[:, :], in_=acc[:])
            nc.scalar.dma_start(out=CH[1], in_=chf["c1"][:])
            nc.scalar.dma_start(out=CH[0], in_=chf["c0"][:])
    nc.compile()
    return nc


_HOST_CTX = {}


def _host_prep(P, weight, bias_abs, bias_q, lambda_abs, lambda_q):
    """Per-core input maps. Host does only O(weights) work plus linear
    passes over P (sum, normalize, transpose, cast)."""
    import ml_dtypes

    s1 = np.arange(QU, dtype=np.float64) / QU
    s0 = np.arange(QL, dtype=np.float64) / QL
    diff2 = (s0[None, :] - s1[:, None]) ** 2            # [l, m]
    t_full = np.expm1(-weight[:, :, None, None].astype(np.float64)
                      * diff2[None, None, :, :]).astype(np.float32)
    sq = s1
    expB = (-bias_q.astype(np.float64) * (sq[None, :] - lambda_q) ** 2
            - bias_abs.astype(np.float64)
            * np.abs(sq[None, :] - lambda_abs))          # [NU, QU] f64

    fp8 = ml_dtypes.float8_e4m3
    P32 = P.astype(np.float32)
    S = P32.sum(axis=2, dtype=np.float64)               # [i, k]
    Pn = (P32 / S[:, :, None]).astype(np.float32)       # P' = P/S
    PT_f8 = Pn.transpose(1, 2, 0).astype(fp8)           # [k, m, i]

    # Group assignment: the dropped 4-way cross term is
    # sum_{a<b in group} r_a r_b with r_k ~ -w[j,k] A_k, so greedily
    # group ks to make the in-group sums of w[:,a]*w[:,b] as negative
    # as possible (pairs first, then pair the pairs).
    M = weight.T @ weight                                # [NL, NL]
    un = list(range(NL))
    pairs = []
    while un:
        a = un.pop(0)
        b = min(un, key=lambda x: M[a, x])
        un.remove(b)
        pairs.append((a, b))
    unp = list(range(len(pairs)))
    perm = []
    while unp:
        p = unp.pop(0)
        a, b = pairs[p]
        q = min(unp, key=lambda q_: M[a, pairs[q_][0]] + M[a, pairs[q_][1]]
                + M[b, pairs[q_][0]] + M[b, pairs[q_][1]])
        unp.remove(q)
        perm.extend(pairs[p] + pairs[q])

    in_maps = []
    eb_rows = []
    for c in range(NCORES):
        tc_ = t_full[c * JLOC:(c + 1) * JLOC]           # [8, k, l, m]
        tc_ = tc_.transpose(1, 3, 0, 2).reshape(NL, QL, JL)  # [k, m, (j,l)]
        # per k: [64 m, 768] rows [P'^T | t']; groups stack 4 ks into
        # 256 contraction slots s, packed DoubleRow-style as [kp, cc]
        # with s = 2*kp + cc; two groups per DMA block
        PTTk = np.empty((NL, QL, PWK), dtype=fp8)
        PTTk[:, :, :B] = PT_f8
        PTTk[:, :, B:] = tc_.astype(fp8)
        PTTk = PTTk[perm]                               # grouping order
        PTTg = PTTk.reshape(NGRP, 2 * KP2, PWK)         # [grp, s, row]
        PTTg = PTTg.reshape(NGRP, KP2, 2 * PWK)         # [grp, kp, cc*row]
        PTTc = np.ascontiguousarray(
            PTTg.reshape(NKB, 2, KP2, 2 * PWK).transpose(0, 2, 1, 3)
            .reshape(NKB, KP2, 4 * PWK))
        eb_rows.append(np.tile(
            expB[c * JLOC:(c + 1) * JLOC].reshape(JL), 2))  # [1024] f64
        in_maps.append({"PTT": PTTc})
    _HOST_CTX["eb"] = eb_rows
    return in_maps


_PROGRAM = None


def _get_program():
    global _PROGRAM
    if _PROGRAM is None:
        _PROGRAM = _build_program()
    return _PROGRAM


def run_on_device(in_maps, trace=False):
    from concourse.bass_utils import run_bass_kernel_spmd
    nc = _get_program()
    return run_bass_kernel_spmd(
        nc, in_maps, core_ids=list(range(NCORES)), trace=trace,
    )


def assemble(results):
    """Host tail in f64: E = c0*c1*exp(acc + sum lg + expB), softmax."""
    out = np.empty((B, NU, QU), dtype=np.float32)
    eb = _HOST_CTX["eb"]
    for c in range(NCORES):
        r = results[c]
        ch = r["ch"].astype(np.float64)                  # [2,128,1024]
        logs = r["acc"].astype(np.float64)               # [128,1024]
        logs += r["lg"].astype(np.float64).sum(axis=0)
        logs += eb[c][None, :]
        E = ch[0] * ch[1] * np.exp(logs)                 # [128,1024]
        E4 = E.reshape(128, 2, JLOC, QU)
        E4 /= E4.sum(axis=3, keepdims=True)
        out[:128, c * JLOC:(c + 1) * JLOC] = E4[:, 0]
        out[128:, c * JLOC:(c + 1) * JLOC] = E4[:, 1]
    return out


def kernel(P, weight, bias_abs, bias_q, lambda_abs, lambda_q):
    in_maps = _host_prep(P, weight, bias_abs, bias_q, lambda_abs, lambda_q)
    res = run_on_device(in_maps, trace=False)
    return assemble(res.results)


# revision 35
# speedup vs baseline: 1.2095x; 1.2095x over previous
"""DRN layer kernel for 8 TRN2 NeuronCores (4-way group-sum + fp8 DoubleRow).

Math (reference):
    T[j,k,l,m]   = exp(-w[j,k] * (s0[m]-s1[l])^2)
    Pw[i,j,k,l]  = sum_m T[j,k,l,m] * P[i,k,m]
    logsum[i,j,l]= sum_k log(Pw[i,j,k,l])
    out          = softmax_l(logsum + exponent_B[j,l])

With P' = P/S and t' = T - 1:  log Pw = log S + log1p(r),
r = sum_m t' P', |r| <= 0.105. log S cancels in the softmax.

Group-sum approximation: sum_{k in G} log1p(r_k) ~= log1p(sum r_k)
for groups of 4 ks (greedy weights-only matching minimizes the dropped
cross terms). Each group's R = sum of 4 r_k comes out of ONE fp8
DoubleRow matmul (256 contraction slots = 4x64 m-rows packed
2-per-cell): 32 MMs per core.

Device does ONLY the matmuls and per-tile consumption:
  c0/c1: DVE product chains     chain = (R + 1) * chain
  g:     ScalarE log1p -> f32, GpSimd adds into an SBUF accumulator
  s:     ScalarE log1p -> fp16, DMA straight to DRAM
exp / exponent_B / softmax normalization run on the HOST (f64), so the
device has no tail: the last consumer op is followed only by its
output DMA. All input DMAs are issued upfront on the sync queue (the
whole 3MB input is SBUF-resident), so no engine queue blocks the feed.

Sharding: tensor-parallel over n_upper: 8 cores x 8 upper nodes, full
batch per core.
"""

import numpy as np

B, NU, NL, QU, QL = 256, 64, 64, 64, 64
NCORES = 8
JLOC = NU // NCORES  # 8 upper nodes per core
JL = JLOC * QU       # 512 = packed (j, l) free dim
NGRP = NL // 4       # 16 k-groups of 4
KP2 = 128            # DoubleRow: 256 contraction slots as [128 part, 2]
PWK = B + JL         # 768 packed width per group: [P'^T (256 i) | t' (512)]
NKB = NGRP // 2      # 8 two-group DMA blocks

# route per group-tile: c0/c1 = DVE product chains, g = ScalarE log +
# GpSimd accumulate (first g writes acc directly), s = ScalarE log ->
# fp16 -> DMA out (host sums the logs). The last tile is 's' so the
# final consumer is the (fast) ScalarE, not the backlogged DVE.
ROUTE = ["g", "c0", "s", "g", "c1", "c0", "g", "c1",
         "c0", "s", "c1", "c0", "s", "c1", "c0", "s"]
NS = ROUTE.count("s")
assert len(ROUTE) == NGRP


def _build_program():
    import concourse.bass as bass
    import concourse.bacc as bacc
    import concourse.mybir as mybir
    from concourse.tile import TileContext

    f32 = mybir.dt.float32
    f16 = mybir.dt.float16
    fp8 = mybir.dt.float8e4
    AF = mybir.ActivationFunctionType
    ALU = mybir.AluOpType

    nc = bacc.Bacc(None, target_bir_lowering=False)
    # raw (non-pool) SBUF scratch for PE warm-up matmuls: deliberately
    # never written -- the values are never consumed, and skipping the
    # memset lets the warm-ups start at tensor-queue boot
    wsrc = nc.alloc_sbuf_tensor("warmsrc", [128, 1280], fp8)
    PTT = nc.declare_dram_parameter("PTT", [NKB, KP2, 4 * PWK], fp8,
                                    isOutput=False)
    CH = nc.declare_dram_parameter("ch", [2, 128, 2 * JL], f16, isOutput=True)
    ACC = nc.declare_dram_parameter("acc", [128, 2 * JL], f16, isOutput=True)
    LG = nc.declare_dram_parameter("lg", [NS, 128, 2 * JL], f16,
                                   isOutput=True)

    with TileContext(nc) as tc:
        with (
            tc.tile_pool(name="pth", bufs=1) as hpool,
            tc.tile_pool(name="ps", bufs=4, space="PSUM") as pspool,
            tc.tile_pool(name="lgw", bufs=2) as lwpool,
            tc.tile_pool(name="lgb", bufs=3) as lbpool,
            tc.tile_pool(name="ch", bufs=1) as chpool,
        ):
            chains = {
                "c0": chpool.tile([128, 2 * JL], f32, tag="ch0", name="ch0"),
                "c1": chpool.tile([128, 2 * JL], f32, tag="ch1", name="ch1"),
            }
            chf = {
                "c0": chpool.tile([128, 2 * JL], f16, tag="cf0", name="cf0"),
                "c1": chpool.tile([128, 2 * JL], f16, tag="cf1", name="cf1"),
            }
            acc = chpool.tile([128, 2 * JL], f32, tag="acc", name="acc")

            # all input DMAs upfront, in ascending-size chunks (in
            # groups: 1,1,2,4,4,4). Small chunks first so the stream
            # starts on the first 192KB; big chunks later mean fewer
            # DMA-completion semaphore waits (~0.5us lag each) at
            # chunk boundaries mid-stream.
            srcs = []  # per group p: (tile, col offset)
            ha = hpool.tile([KP2, 2 * PWK], fp8, tag="ptta")
            nc.sync.dma_start(out=ha[:], in_=PTT[0, :, :2 * PWK])
            hb = hpool.tile([KP2, 2 * PWK], fp8, tag="pttb")
            nc.sync.dma_start(out=hb[:], in_=PTT[0, :, 2 * PWK:])
            srcs += [(ha, 0), (hb, 0)]
            for kb in range(1, NKB):
                ptt = hpool.tile([KP2, 4 * PWK], fp8, tag=f"ptt{kb}")
                nc.sync.dma_start(out=ptt[:], in_=PTT[kb])
                srcs += [(ptt, 0), (ptt, 2 * PWK)]

            # PE pstate warm-up: the tensor engine only reaches its full
            # clock after ~3us of sustained execution, so burn the DMA
            # pre-roll on dummy matmuls. Their PSUM tile is a regular
            # ring slot reused by the real loop.
            wv = wsrc.ap().rearrange("q (c w) -> q c w", c=2)
            warm = pspool.tile([128, 2 * JL], f32, tag="ps", name="ps")
            for _ in range(8):
                nc.tensor.matmul(
                    warm[:, :JL], lhsT=wv[:, :, :128], rhs=wv[:, :, 128:],
                    start=True, stop=True,
                    perf_mode=mybir.MatmulPerfMode.DoubleRow)

            started = {"c0": False, "c1": False, "g": False}
            si = 0
            exports = []
            for p in range(NGRP):
                tile, off = srcs[p]
                pk = tile[:, off:off + 2 * PWK].rearrange(
                    "q (c w) -> q c w", c=2)
                ps = pspool.tile([128, 2 * JL], f32, tag="ps", name="ps")
                for ih in range(2):
                    nc.tensor.matmul(
                        ps[:, ih * JL:(ih + 1) * JL],
                        lhsT=pk[:, :, ih * 128:(ih + 1) * 128],
                        rhs=pk[:, :, B:PWK],
                        start=True, stop=True,
                        perf_mode=mybir.MatmulPerfMode.DoubleRow)
                r = ROUTE[p]
                if r in ("c0", "c1"):
                    ch = chains[r]
                    last = p == max(i for i, x in enumerate(ROUTE) if x == r)
                    if not started[r]:
                        nc.vector.tensor_scalar_add(ch[:], ps[:], 1.0)
                        started[r] = True
                    elif last:  # final factor writes the f16 export tile
                        nc.vector.scalar_tensor_tensor(
                            chf[r][:], ps[:], 1.0, ch[:],
                            op0=ALU.add, op1=ALU.mult)
                    else:
                        nc.vector.scalar_tensor_tensor(
                            ch[:], ps[:], 1.0, ch[:],
                            op0=ALU.add, op1=ALU.mult)
                elif r == "g":
                    if not started["g"]:
                        nc.scalar.activation(acc[:], ps[:], AF.Ln, bias=1.0)
                        started["g"] = True
                    else:
                        lgw = lwpool.tile([128, 2 * JL], f32, tag="lgw",
                                          name="lgw")
                        nc.scalar.activation(lgw[:], ps[:], AF.Ln, bias=1.0)
                        nc.gpsimd.tensor_add(acc[:], acc[:], lgw[:])
                else:  # s: log1p -> fp16 -> DRAM, host sums
                    lgb = lbpool.tile([128, 2 * JL], f16, tag="lgb",
                                      name="lgb")
                    nc.scalar.activation(lgb[:], ps[:], AF.Ln, bias=1.0)
                    nc.scalar.dma_start(out=LG[si], in_=lgb[:])
                    si += 1

            # Exports NEVER go on the sync queue: the SP sequencer also
            # processes every DMA-completion semaphore event in order, so
            # a waiting export trigger would head-of-line block the
            # events the matmul stream depends on. acc + c0 ride the
            # gpsimd SWDGE (idle; acc is cast f32->f16), c1 rides scalar
            # after the last LN.
            nc.gpsimd.dma_start(out=ACC[:, :], in_=acc[:])
            nc.scalar.dma_start(out=CH[1], in_=chf["c1"][:])
            nc.scalar.dma_start(out=CH[0], in_=chf["c0"][:])
    nc.compile()
    return nc


_HOST_CTX = {}


def _host_prep(P, weight, bias_abs, bias_q, lambda_abs, lambda_q):
    """Per-core input maps. Host does only O(weights) work plus linear
    passes over P (sum, normalize, transpose, cast)."""
    import ml_dtypes

    s1 = np.arange(QU, dtype=np.float64) / QU
    s0 = np.arange(QL, dtype=np.float64) / QL
    diff2 = (s0[None, :] - s1[:, None]) ** 2            # [l, m]
    t_full = np.expm1(-weight[:, :, None, None].astype(np.float64)
                      * diff2[None, None, :, :]).astype(np.float32)
    sq = s1
    expB = (-bias_q.astype(np.float64) * (sq[None, :] - lambda_q) ** 2
            - bias_abs.astype(np.float64)
            * np.abs(sq[None, :] - lambda_abs))          # [NU, QU] f64

    fp8 = ml_dtypes.float8_e4m3
    P32 = P.astype(np.float32)
    S = P32.sum(axis=2, dtype=np.float64)               # [i, k]
    Pn = (P32 / S[:, :, None]).astype(np.float32)       # P' = P/S
    PT_f8 = Pn.transpose(1, 2, 0).astype(fp8)           # [k, m, i]

    # Group assignment: the dropped 4-way cross term is
    # sum_{a<b in group} r_a r_b with r_k ~ -w[j,k] A_k, so greedily
    # group ks to make the in-group sums of w[:,a]*w[:,b] as negative
    # as possible (pairs first, then pair the pairs).
    M = weight.T @ weight                                # [NL, NL]
    un = list(range(NL))
    pairs = []
    while un:
        a = un.pop(0)
        b = min(un, key=lambda x: M[a, x])
        un.remove(b)
        pairs.append((a, b))
    unp = list(range(len(pairs)))
    perm = []
    while unp:
        p = unp.pop(0)
        a, b = pairs[p]
        q = min(unp, key=lambda q_: M[a, pairs[q_][0]] + M[a, pairs[q_][1]]
                + M[b, pairs[q_][0]] + M[b, pairs[q_][1]])
        unp.remove(q)
        perm.extend(pairs[p] + pairs[q])

    in_maps = []
    eb_rows = []
    for c in range(NCORES):
        tc_ = t_full[c * JLOC:(c + 1) * JLOC]           # [8, k, l, m]
        tc_ = tc_.transpose(1, 3, 0, 2).reshape(NL, QL, JL)  # [k, m, (j,l)]
        # per k: [64 m, 768] rows [P'^T | t']; groups stack 4 ks into
        # 256 contraction slots s, packed DoubleRow-style as [kp, cc]
        # with s = 2*kp + cc; two groups per DMA block
        PTTk = np.empty((NL, QL, PWK), dtype=fp8)
        PTTk[:, :, :B] = PT_f8
        PTTk[:, :, B:] = tc_.astype(fp8)
        PTTk = PTTk[perm]                               # grouping order
        PTTg = PTTk.reshape(NGRP, 2 * KP2, PWK)         # [grp, s, row]
        PTTg = PTTg.reshape(NGRP, KP2, 2 * PWK)         # [grp, kp, cc*row]
        PTTc = np.ascontiguousarray(
            PTTg.reshape(NKB, 2, KP2, 2 * PWK).transpose(0, 2, 1, 3)
            .reshape(NKB, KP2, 4 * PWK))
        eb_rows.append(np.tile(
            expB[c * JLOC:(c + 1) * JLOC].reshape(JL), 2))  # [1024] f64
        in_maps.append({"PTT": PTTc})
    _HOST_CTX["eb"] = eb_rows
    return in_maps


_PROGRAM = None


def _get_program():
    global _PROGRAM
    if _PROGRAM is None:
        _PROGRAM = _build_program()
    return _PROGRAM


def run_on_device(in_maps, trace=False):
    from concourse.bass_utils import run_bass_kernel_spmd
    nc = _get_program()
    return run_bass_kernel_spmd(
        nc, in_maps, core_ids=list(range(NCORES)), trace=trace,
    )


def assemble(results):
    """Host tail in f64: E = c0*c1*exp(acc + sum lg + expB), softmax."""
    out = np.empty((B, NU, QU), dtype=np.float32)
    eb = _HOST_CTX["eb"]
    for c in range(NCORES):
        r = results[c]
        ch = r["ch"].astype(np.float64)                  # [2,128,1024]
        logs = r["acc"].astype(np.float64)               # [128,1024]
        logs += r["lg"].astype(np.float64).sum(axis=0)
        logs += eb[c][None, :]
        E = ch[0] * ch[1] * np.exp(logs)                 # [128,1024]
        E4 = E.reshape(128, 2, JLOC, QU)
        E4 /= E4.sum(axis=3, keepdims=True)
        out[:128, c * JLOC:(c + 1) * JLOC] = E4[:, 0]
        out[128:, c * JLOC:(c + 1) * JLOC] = E4[:, 1]
    return out


def kernel(P, weight, bias_abs, bias_q, lambda_abs, lambda_q):
    in_maps = _host_prep(P, weight, bias_abs, bias_q, lambda_abs, lambda_q)
    res = run_on_device(in_maps, trace=False)
    return assemble(res.results)
